# revision 2
# baseline (speedup 1.0000x reference)
"""Trainium2 Bass kernel for nn_ModalMoE: concat -> shared gelu MLP -> softmax top-2 gate
-> 8-expert gelu MoE combine.

Data-parallel over the batch across 8 NeuronCores (weights replicated).

Self-contained: hardcodes shapes; only imports concourse from /opt/trn_rl_repo.
"""
import sys

sys.path.insert(0, "/opt/trn_rl_repo")

import numpy as np
from concourse import bacc, tile, bass, bass_utils
import concourse.mybir as mybir

dt = mybir.dt
AF = mybir.ActivationFunctionType
ALU = mybir.AluOpType

N_CORES = 8
B = 16384
T = B // N_CORES          # tokens per core (2048)
NT = T // 128             # 128-token tiles per core (16)
NB = T // 512             # 512-token blocks per core (4)
F = 1536                  # concat feature dim
KF = F // 128             # 12 feature chunks
D = 1024
KD = D // 128             # 8 d chunks
E = 8
F0, F1, F2 = 768, 512, 256

EXPERT_F32R = True        # expert matmuls in f32r (else fp32)


def build_kernel(has_b_gate: bool, has_b_experts: bool):
    nc = bacc.Bacc("TRN2", target_bir_lowering=False)

    feat0 = nc.dram_tensor("feat0", [T, F0], dt.float32, kind="ExternalInput").ap()
    feat1 = nc.dram_tensor("feat1", [T, F1], dt.float32, kind="ExternalInput").ap()
    feat2 = nc.dram_tensor("feat2", [T, F2], dt.float32, kind="ExternalInput").ap()
    W_shared = nc.dram_tensor("W_shared", [F, D], dt.float32, kind="ExternalInput").ap()
    b_shared = nc.dram_tensor("b_shared", [D], dt.float32, kind="ExternalInput").ap()
    W_gate = nc.dram_tensor("W_gate", [D, E], dt.float32, kind="ExternalInput").ap()
    b_gate = nc.dram_tensor("b_gate", [E], dt.float32, kind="ExternalInput").ap()
    W_experts = nc.dram_tensor("W_experts", [E, D, D], dt.float32, kind="ExternalInput").ap()
    b_experts = nc.dram_tensor("b_experts", [E, D], dt.float32, kind="ExternalInput").ap()
    ident_in = nc.dram_tensor("ident", [128, 128], dt.float32, kind="ExternalInput").ap()
    out = nc.dram_tensor("out", [T, D], dt.float32, kind="ExternalOutput").ap()

    ex_dt = dt.float32r if EXPERT_F32R else dt.float32

    with tile.TileContext(nc) as tc:
        with tc.tile_pool(name="persist", bufs=1) as persist:
            ident = persist.tile([128, 128], dt.float32)
            nc.sync.dma_start(ident[:], ident_in)
            ones_row = persist.tile([1, 128], dt.float32)
            nc.vector.memset(ones_row[:], 1.0)
            b_sh = persist.tile([128, KD], dt.float32)
            nc.sync.dma_start(b_sh[:], b_shared.rearrange("(k p) -> p k", p=128))
            wg_sb = persist.tile([128, KD, E], dt.float32)
            nc.sync.dma_start(wg_sb[:], W_gate.rearrange("(k p) e -> p k e", p=128))
            if has_b_gate:
                bg_sb = persist.tile([1, E], dt.float32)
                nc.sync.dma_start(bg_sb[:], b_gate[None, :])
            if has_b_experts:
                be_sb = persist.tile([1, E, D], dt.float32)
                nc.sync.dma_start(be_sb[:], b_experts[None, :, :])

            wgt = persist.tile([128, NT, E], dt.float32)   # gating weights per token
            hT_r = persist.tile([128, KD, T], ex_dt)       # h transposed [d, tok] rounded

            # ---- Phase 1+2: hT = gelu(x @ W_shared + b); gate softmax top-2 per block
            with (
                tc.tile_pool(name="p1w", bufs=1) as p1w,
                tc.tile_pool(name="p1", bufs=1) as p1,
                tc.tile_pool(name="p1s", bufs=2) as p1s,
                tc.tile_pool(name="p2", bufs=2) as p2,
                tc.tile_pool(name="psum_h", bufs=2, space="PSUM") as psum_h,
                tc.tile_pool(name="psum_t", bufs=2, space="PSUM") as psum_t,
                tc.tile_pool(name="psum_g", bufs=2, space="PSUM") as psum_g,
            ):
                wsh = p1w.tile([128, KF, D], dt.float32, tag="wsh")
                nc.sync.dma_start(wsh[:], W_shared.rearrange("(k p) d -> p k d", p=128))

                for b in range(NB):
                    xT = p1.tile([128, KF, 512], dt.float32, tag="xT")
                    for tt in range(4):
                        t = b * 4 + tt
                        xs = p1s.tile([128, F], dt.float32, tag="xs")
                        nc.sync.dma_start(xs[:, 0:F0], feat0[t * 128:(t + 1) * 128, :])
                        nc.sync.dma_start(xs[:, F0:F0 + F1], feat1[t * 128:(t + 1) * 128, :])
                        nc.sync.dma_start(xs[:, F0 + F1:F], feat2[t * 128:(t + 1) * 128, :])
                        for k in range(KF):
                            pt = psum_t.tile([128, 128], dt.float32, tag="pt")
                            nc.tensor.transpose(pt[:], xs[:, k * 128:(k + 1) * 128], ident[:])
                            nc.vector.tensor_copy(xT[:, k, tt * 128:(tt + 1) * 128], pt[:])
                    hb = p1.tile([128, KD, 512], dt.float32, tag="hb")
                    for dk in range(KD):
                        ph = psum_h.tile([128, 512], dt.float32, tag="ph")
                        for k in range(KF):
                            nc.tensor.matmul(
                                ph[:], wsh[:, k, dk * 128:(dk + 1) * 128], xT[:, k, :],
                                start=(k == 0), stop=(k == KF - 1),
                            )
                        nc.scalar.activation(hb[:, dk, :], ph[:], AF.Gelu,
                                             bias=b_sh[:, dk:dk + 1])
                        nc.vector.tensor_copy(hT_r[:, dk, b * 512:(b + 1) * 512],
                                              hb[:, dk, :])
                    # gate for this block's 4 token tiles (fp32 exact)
                    for tt in range(4):
                        t = b * 4 + tt
                        pg = psum_g.tile([128, E], dt.float32, tag="pg")
                        if has_b_gate:
                            nc.tensor.matmul(pg[:], ones_row[:], bg_sb[:],
                                             start=True, stop=False)
                        for k in range(KD):
                            nc.tensor.matmul(
                                pg[:], hb[:, k, tt * 128:(tt + 1) * 128], wg_sb[:, k, :],
                                start=(k == 0 and not has_b_gate), stop=(k == KD - 1),
                            )
                        lg = p2.tile([128, E], dt.float32, tag="lg")
                        nc.vector.tensor_copy(lg[:], pg[:])
                        m1n = p2.tile([128, 1], dt.float32, tag="m1n")
                        nc.vector.tensor_reduce(m1n[:], lg[:], axis=mybir.AxisListType.X,
                                                op=ALU.max, negate=True)
                        ex = p2.tile([128, E], dt.float32, tag="ex")
                        nc.scalar.activation(ex[:], lg[:], AF.Exp, bias=m1n[:])
                        z = p2.tile([128, 1], dt.float32, tag="z")
                        nc.vector.tensor_reduce(z[:], ex[:], axis=mybir.AxisListType.X,
                                                op=ALU.add)
                        zr = p2.tile([128, 1], dt.float32, tag="zr")
                        nc.vector.reciprocal(zr[:], z[:])
                        eq = p2.tile([128, E], dt.float32, tag="eq")
                        nc.vector.tensor_scalar(eq[:], lg[:], m1n[:], 0.0,
                                                op0=ALU.add, op1=ALU.is_ge)
                        tmp = p2.tile([128, E], dt.float32, tag="tmp")
                        nc.vector.scalar_tensor_tensor(tmp[:], eq[:], -1e30, lg[:],
                                                       op0=ALU.mult, op1=ALU.add)
                        m2n = p2.tile([128, 1], dt.float32, tag="m2n")
                        nc.vector.tensor_reduce(m2n[:], tmp[:], axis=mybir.AxisListType.X,
                                                op=ALU.max, negate=True)
                        mask = p2.tile([128, E], dt.float32, tag="mask")
                        nc.vector.tensor_scalar(mask[:], lg[:], m2n[:], 0.0,
                                                op0=ALU.add, op1=ALU.is_ge)
                        nc.vector.scalar_tensor_tensor(wgt[:, t, :], ex[:], zr[:], mask[:],
                                                       op0=ALU.mult, op1=ALU.mult)

            # ---- Phase 3: dense experts, weighted accumulate
            with (
                tc.tile_pool(name="p3acc", bufs=1) as p3acc,
                tc.tile_pool(name="p3", bufs=2) as p3,
                tc.tile_pool(name="p3s", bufs=1) as p3s,
                tc.tile_pool(name="psum_e", bufs=4, space="PSUM") as psum_e,
            ):
                acc = p3acc.tile([128, NT, D], dt.float32)
                for e in range(E):
                    for half in range(2):
                        ws = p3s.tile([128, KD, 512], dt.float32, tag="ws")
                        nc.sync.dma_start(
                            ws[:],
                            W_experts[e].rearrange("(k p) d -> p k d", p=128)[
                                :, :, half * 512:(half + 1) * 512],
                        )
                        if EXPERT_F32R:
                            wr = p3.tile([128, KD, 512], dt.float32r, tag="wr")
                            for k in range(KD):
                                nc.vector.tensor_copy(wr[:, k, :], ws[:, k, :])
                        else:
                            wr = ws
                        for t in range(NT):
                            pe_ = psum_e.tile([128, 512], dt.float32, tag="pe")
                            if has_b_experts:
                                nc.tensor.matmul(
                                    pe_[:], ones_row[:],
                                    be_sb[:, e, half * 512:(half + 1) * 512],
                                    start=True, stop=False)
                            for k in range(KD):
                                nc.tensor.matmul(
                                    pe_[:], hT_r[:, k, t * 128:(t + 1) * 128],
                                    wr[:, k, :],
                                    start=(k == 0 and not has_b_experts),
                                    stop=(k == KD - 1),
                                )
                            g = p3.tile([128, 512], dt.float32, tag="g")
                            nc.scalar.activation(g[:], pe_[:], AF.Gelu)
                            nc.vector.scalar_tensor_tensor(
                                acc[:, t, half * 512:(half + 1) * 512],
                                g[:], wgt[:, t, e:e + 1],
                                acc[:, t, half * 512:(half + 1) * 512],
                                op0=ALU.mult,
                                op1=ALU.bypass if e == 0 else ALU.add,
                            )
                nc.sync.dma_start(out.rearrange("(t p) d -> p t d", p=128), acc[:])

    nc.compile()
    return nc


_nc_cache = {}


def _get_nc(has_b_gate, has_b_experts):
    key = (has_b_gate, has_b_experts, EXPERT_F32R)
    if key not in _nc_cache:
        _nc_cache[key] = build_kernel(has_b_gate, has_b_experts)
    return _nc_cache[key]


def kernel(feat0, feat1, feat2, W_shared, b_shared, W_gate, b_gate, W_experts, b_experts):
    feat0 = np.ascontiguousarray(feat0, dtype=np.float32)
    feat1 = np.ascontiguousarray(feat1, dtype=np.float32)
    feat2 = np.ascontiguousarray(feat2, dtype=np.float32)
    has_b_gate = bool(np.any(b_gate))
    has_b_experts = bool(np.any(b_experts))
    nc = _get_nc(has_b_gate, has_b_experts)
    ident = np.eye(128, dtype=np.float32)
    shared = {
        "W_shared": np.ascontiguousarray(W_shared, np.float32),
        "b_shared": np.ascontiguousarray(b_shared, np.float32),
        "W_gate": np.ascontiguousarray(W_gate, np.float32),
        "b_gate": np.ascontiguousarray(b_gate, np.float32),
        "W_experts": np.ascontiguousarray(W_experts, np.float32),
        "b_experts": np.ascontiguousarray(b_experts, np.float32),
        "ident": ident,
    }
    in_maps = []
    for c in range(N_CORES):
        sl = slice(c * T, (c + 1) * T)
        m = dict(shared)
        m["feat0"] = feat0[sl]
        m["feat1"] = feat1[sl]
        m["feat2"] = feat2[sl]
        in_maps.append(m)
    res = bass_utils.run_bass_kernel_spmd(nc, in_maps, core_ids=list(range(N_CORES)))
    return np.concatenate([res.results[c]["out"] for c in range(N_CORES)], axis=0)


# revision 8
# speedup vs baseline: 67.1031x; 67.1031x over previous
"""Trainium2 Bass kernel for nn_ModalMoE: concat -> shared gelu MLP -> softmax top-2 gate
-> 8-expert gelu MoE combine.

Data-parallel over the batch across 8 NeuronCores (weights replicated).

Self-contained: hardcodes shapes; only imports concourse from /opt/trn_rl_repo.
"""
import sys

sys.path.insert(0, "/opt/trn_rl_repo")

import numpy as np
from concourse import bacc, tile, bass, bass_utils
import concourse.mybir as mybir

dt = mybir.dt
AF = mybir.ActivationFunctionType
ALU = mybir.AluOpType

N_CORES = 8
B = 16384
T = B // N_CORES          # tokens per core (2048)
NT = T // 128             # 128-token tiles per core (16)
NB = T // 512             # 512-token blocks per core (4)
F = 1536                  # concat feature dim
KF = F // 128             # 12 feature chunks
D = 1024
KD = D // 128             # 8 d chunks
E = 8
F0, F1, F2 = 768, 512, 256

EXPERT_F32R = True        # expert matmuls in f32r (else fp32)
SHARED_MODE = "f32"       # "f32" | "f32r" | "bf16x3"  (shared-layer matmul mode)


def build_kernel(has_b_gate: bool, has_b_experts: bool, repeat: int = 1):
    nc = bacc.Bacc("TRN2", target_bir_lowering=False)

    feat0 = nc.dram_tensor("feat0", [T, F0], dt.float32, kind="ExternalInput").ap()
    feat1 = nc.dram_tensor("feat1", [T, F1], dt.float32, kind="ExternalInput").ap()
    feat2 = nc.dram_tensor("feat2", [T, F2], dt.float32, kind="ExternalInput").ap()
    W_shared = nc.dram_tensor("W_shared", [F, D], dt.float32, kind="ExternalInput").ap()
    b_shared = nc.dram_tensor("b_shared", [D], dt.float32, kind="ExternalInput").ap()
    W_gate = nc.dram_tensor("W_gate", [D, E], dt.float32, kind="ExternalInput").ap()
    b_gate = nc.dram_tensor("b_gate", [E], dt.float32, kind="ExternalInput").ap()
    W_experts = nc.dram_tensor("W_experts", [E, D, D],
                               dt.float32r if EXPERT_F32R else dt.float32,
                               kind="ExternalInput").ap()
    b_experts = nc.dram_tensor("b_experts", [E, D], dt.float32, kind="ExternalInput").ap()
    ident_in = nc.dram_tensor("ident", [128, 128], dt.float32, kind="ExternalInput").ap()
    out = nc.dram_tensor("out", [T, D], dt.float32, kind="ExternalOutput").ap()

    ex_dt = dt.float32r if EXPERT_F32R else dt.float32
    sh_dt = dt.float32r if SHARED_MODE == "f32r" else dt.float32

    with tile.TileContext(nc) as tc:
      for _rep in range(repeat):
        with tc.tile_pool(name="persist", bufs=1) as persist:
            ident = persist.tile([128, 128], dt.float32)
            nc.sync.dma_start(ident[:], ident_in)
            ones_row = persist.tile([1, 128], dt.float32)
            nc.vector.memset(ones_row[:], 1.0)
            b_sh = persist.tile([128, KD], dt.float32)
            nc.sync.dma_start(b_sh[:], b_shared.rearrange("(k p) -> p k", p=128))
            wg_sb = persist.tile([128, KD, E], dt.float32)
            nc.sync.dma_start(wg_sb[:], W_gate.rearrange("(k p) e -> p k e", p=128))
            if has_b_gate:
                bg_sb = persist.tile([1, E], dt.float32)
                nc.sync.dma_start(bg_sb[:], b_gate[None, :])
            if has_b_experts:
                be_sb = persist.tile([1, E, D], dt.float32)
                nc.sync.dma_start(be_sb[:], b_experts[None, :, :])

            wgt = persist.tile([128, NT, E], dt.float32)   # gating weights per token
            hT_r = persist.tile([128, KD, T], ex_dt)       # h transposed [d, tok] rounded

            # ---- Phase 1+2: hT = gelu(x @ W_shared + b); gate softmax top-2 per block
            with (
                tc.tile_pool(name="p1w", bufs=1) as p1w,
                tc.tile_pool(name="p1", bufs=1) as p1,
                tc.tile_pool(name="p1s", bufs=2) as p1s,
                tc.tile_pool(name="p2", bufs=2) as p2,
                tc.tile_pool(name="psum_h", bufs=2, space="PSUM") as psum_h,
                tc.tile_pool(name="psum_t", bufs=2, space="PSUM") as psum_t,
                tc.tile_pool(name="psum_g", bufs=2, space="PSUM") as psum_g,
            ):
                wview = W_shared.rearrange("(k p) d -> p k d", p=128)
                if SHARED_MODE == "f32r":
                    wsh_mm = p1w.tile([128, KF, D], dt.float32r, tag="wshr")
                    for k in range(KF):
                        wst = p1s.tile([128, D], dt.float32, tag="wst")
                        nc.sync.dma_start(wst[:], wview[:, k, :])
                        nc.vector.tensor_copy(wsh_mm[:, k, :], wst[:])
                elif SHARED_MODE == "bf16x3":
                    wsh_h = p1w.tile([128, KF, D], dt.bfloat16, tag="wshh")
                    wsh_l = p1w.tile([128, KF, D], dt.bfloat16, tag="wshl")
                    for k in range(KF):
                        wst = p1s.tile([128, D], dt.float32, tag="wst")
                        nc.sync.dma_start(wst[:], wview[:, k, :])
                        nc.vector.tensor_copy(wsh_h[:, k, :], wst[:])
                        nc.vector.scalar_tensor_tensor(
                            wsh_l[:, k, :], wst[:], 0.0, wsh_h[:, k, :],
                            op0=ALU.bypass, op1=ALU.subtract)
                else:
                    wsh_mm = p1w.tile([128, KF, D], dt.float32, tag="wsh")
                    nc.sync.dma_start(wsh_mm[:], wview)

                for b in range(NB):
                    if SHARED_MODE == "bf16x3":
                        xTh = p1.tile([128, KF, 512], dt.bfloat16, tag="xTh")
                        xTl = p1.tile([128, KF, 512], dt.bfloat16, tag="xTl")
                    else:
                        xT = p1.tile([128, KF, 512], sh_dt, tag="xT")
                    for tt in range(4):
                        t = b * 4 + tt
                        xs = p1s.tile([128, F], dt.float32, tag="xs")
                        nc.sync.dma_start(xs[:, 0:F0], feat0[t * 128:(t + 1) * 128, :])
                        nc.sync.dma_start(xs[:, F0:F0 + F1], feat1[t * 128:(t + 1) * 128, :])
                        nc.sync.dma_start(xs[:, F0 + F1:F], feat2[t * 128:(t + 1) * 128, :])
                        sl = slice(tt * 128, (tt + 1) * 128)
                        for kg in range(KF // 4):
                            pt = psum_t.tile([128, 4, 128], dt.float32, tag="pt")
                            for j in range(4):
                                k = kg * 4 + j
                                nc.tensor.transpose(pt[:, j, :],
                                                    xs[:, k * 128:(k + 1) * 128], ident[:])
                            ksl = slice(kg * 4, kg * 4 + 4)
                            if SHARED_MODE == "bf16x3":
                                nc.vector.tensor_copy(xTh[:, ksl, sl], pt[:])
                                nc.vector.scalar_tensor_tensor(
                                    xTl[:, ksl, sl], pt[:], 0.0, xTh[:, ksl, sl],
                                    op0=ALU.bypass, op1=ALU.subtract)
                            else:
                                nc.vector.tensor_copy(xT[:, ksl, sl], pt[:])
                    hb = p1.tile([128, KD, 512], dt.float32, tag="hb")
                    for dk in range(KD):
                        ph = psum_h.tile([128, 512], dt.float32, tag="ph")
                        if SHARED_MODE == "bf16x3":
                            dsl = slice(dk * 128, (dk + 1) * 128)
                            for k in range(KF):
                                nc.tensor.matmul(ph[:], wsh_h[:, k, dsl], xTh[:, k, :],
                                                 start=(k == 0), stop=False)
                            for k in range(KF):
                                nc.tensor.matmul(ph[:], wsh_l[:, k, dsl], xTh[:, k, :],
                                                 start=False, stop=False)
                            for k in range(KF):
                                nc.tensor.matmul(ph[:], wsh_h[:, k, dsl], xTl[:, k, :],
                                                 start=False, stop=(k == KF - 1))
                        else:
                            for k in range(KF):
                                nc.tensor.matmul(
                                    ph[:], wsh_mm[:, k, dk * 128:(dk + 1) * 128], xT[:, k, :],
                                    start=(k == 0), stop=(k == KF - 1),
                                )
                        nc.scalar.activation(hb[:, dk, :], ph[:], AF.Gelu,
                                             bias=b_sh[:, dk:dk + 1])
                        nc.vector.tensor_copy(hT_r[:, dk, b * 512:(b + 1) * 512],
                                              hb[:, dk, :])
                    # gate for this block's 4 token tiles (fp32 exact)
                    for tt in range(4):
                        t = b * 4 + tt
                        pg = psum_g.tile([128, E], dt.float32, tag="pg")
                        if has_b_gate:
                            nc.tensor.matmul(pg[:], ones_row[:], bg_sb[:],
                                             start=True, stop=False)
                        for k in range(KD):
                            nc.tensor.matmul(
                                pg[:], hb[:, k, tt * 128:(tt + 1) * 128], wg_sb[:, k, :],
                                start=(k == 0 and not has_b_gate), stop=(k == KD - 1),
                            )
                        lg = p2.tile([128, E], dt.float32, tag="lg")
                        nc.vector.tensor_copy(lg[:], pg[:])
                        m1n = p2.tile([128, 1], dt.float32, tag="m1n")
                        nc.vector.tensor_reduce(m1n[:], lg[:], axis=mybir.AxisListType.X,
                                                op=ALU.max, negate=True)
                        ex = p2.tile([128, E], dt.float32, tag="ex")
                        nc.scalar.activation(ex[:], lg[:], AF.Exp, bias=m1n[:])
                        z = p2.tile([128, 1], dt.float32, tag="z")
                        nc.vector.tensor_reduce(z[:], ex[:], axis=mybir.AxisListType.X,
                                                op=ALU.add)
                        zr = p2.tile([128, 1], dt.float32, tag="zr")
                        nc.vector.reciprocal(zr[:], z[:])
                        eq = p2.tile([128, E], dt.float32, tag="eq")
                        nc.vector.tensor_scalar(eq[:], lg[:], m1n[:], 0.0,
                                                op0=ALU.add, op1=ALU.is_ge)
                        tmp = p2.tile([128, E], dt.float32, tag="tmp")
                        nc.vector.scalar_tensor_tensor(tmp[:], eq[:], -1e30, lg[:],
                                                       op0=ALU.mult, op1=ALU.add)
                        m2n = p2.tile([128, 1], dt.float32, tag="m2n")
                        nc.vector.tensor_reduce(m2n[:], tmp[:], axis=mybir.AxisListType.X,
                                                op=ALU.max, negate=True)
                        mask = p2.tile([128, E], dt.float32, tag="mask")
                        nc.vector.tensor_scalar(mask[:], lg[:], m2n[:], 0.0,
                                                op0=ALU.add, op1=ALU.is_ge)
                        nc.vector.scalar_tensor_tensor(wgt[:, t, :], ex[:], zr[:], mask[:],
                                                       op0=ALU.mult, op1=ALU.mult)

            # ---- Phase 3: dense experts, weighted accumulate
            with (
                tc.tile_pool(name="p3acc", bufs=1) as p3acc,
                tc.tile_pool(name="p3", bufs=2) as p3,
                tc.tile_pool(name="p3w", bufs=3) as p3w,
                tc.tile_pool(name="psum_e", bufs=6, space="PSUM") as psum_e,
            ):
                acc = p3acc.tile([128, NT, D], dt.float32)
                for e in range(E):
                    for half in range(2):
                        wr = p3w.tile([128, KD, 512],
                                      dt.float32r if EXPERT_F32R else dt.float32,
                                      tag="wr")
                        nc.sync.dma_start(
                            wr[:],
                            W_experts[e].rearrange("(k p) d -> p k d", p=128)[
                                :, :, half * 512:(half + 1) * 512],
                        )
                        for t in range(NT):
                            pe_ = psum_e.tile([128, 512], dt.float32, tag="pe")
                            if has_b_experts:
                                nc.tensor.matmul(
                                    pe_[:], ones_row[:],
                                    be_sb[:, e, half * 512:(half + 1) * 512],
                                    start=True, stop=False)
                            for k in range(KD):
                                nc.tensor.matmul(
                                    pe_[:], hT_r[:, k, t * 128:(t + 1) * 128],
                                    wr[:, k, :],
                                    start=(k == 0 and not has_b_experts),
                                    stop=(k == KD - 1),
                                )
                            g = p3.tile([128, 512], dt.float32, tag="g")
                            nc.scalar.activation(g[:], pe_[:], AF.Gelu)
                            nc.vector.scalar_tensor_tensor(
                                acc[:, t, half * 512:(half + 1) * 512],
                                g[:], wgt[:, t, e:e + 1],
                                acc[:, t, half * 512:(half + 1) * 512],
                                op0=ALU.mult,
                                op1=ALU.bypass if e == 0 else ALU.add,
                            )
                nc.sync.dma_start(out.rearrange("(t p) d -> p t d", p=128), acc[:])

    nc.compile()
    return nc


_nc_cache = {}


def _get_nc(has_b_gate, has_b_experts, repeat=1):
    key = (has_b_gate, has_b_experts, EXPERT_F32R, SHARED_MODE, repeat)
    if key not in _nc_cache:
        _nc_cache[key] = build_kernel(has_b_gate, has_b_experts, repeat)
    return _nc_cache[key]


def kernel(feat0, feat1, feat2, W_shared, b_shared, W_gate, b_gate, W_experts, b_experts):
    feat0 = np.ascontiguousarray(feat0, dtype=np.float32)
    feat1 = np.ascontiguousarray(feat1, dtype=np.float32)
    feat2 = np.ascontiguousarray(feat2, dtype=np.float32)
    has_b_gate = bool(np.any(b_gate))
    has_b_experts = bool(np.any(b_experts))
    nc = _get_nc(has_b_gate, has_b_experts)
    ident = np.eye(128, dtype=np.float32)
    W_experts = np.ascontiguousarray(W_experts, np.float32)
    if EXPERT_F32R:
        u = W_experts.view(np.uint32).astype(np.uint64)
        bias = ((u >> 12) & 1) + 0x7FF
        W_experts = (((u + bias) >> 12) << 12).astype(np.uint32).view(np.float32)
    shared = {
        "W_shared": np.ascontiguousarray(W_shared, np.float32),
        "b_shared": np.ascontiguousarray(b_shared, np.float32),
        "W_gate": np.ascontiguousarray(W_gate, np.float32),
        "b_gate": np.ascontiguousarray(b_gate, np.float32),
        "W_experts": W_experts,
        "b_experts": np.ascontiguousarray(b_experts, np.float32),
        "ident": ident,
    }
    in_maps = []
    for c in range(N_CORES):
        sl = slice(c * T, (c + 1) * T)
        m = dict(shared)
        m["feat0"] = feat0[sl]
        m["feat1"] = feat1[sl]
        m["feat2"] = feat2[sl]
        in_maps.append(m)
    res = bass_utils.run_bass_kernel_spmd(nc, in_maps, core_ids=list(range(N_CORES)))
    return np.concatenate([res.results[c]["out"] for c in range(N_CORES)], axis=0)
